# revision 32
# baseline (speedup 1.0000x reference)
"""Trainium2 8-core kernel for the ConvFF + BatchNorm + Mamba block.

Sharding (8 NeuronCores): core i -> b = i//4 (batch), q = i%4.
  - Front (ff conv + BN): computes output-channel tile q (128 of 512
    channels) for batch b. BN stats all-reduced across the b-pair.
  - normed all-gathered within each b-group of 4 cores.
  - Mamba: d_inner slice q (256 of 1024 channels) for batch b; the
    x_proj partial is all-reduced within the b-group; the selective
    scan runs fully local via the DVE tensor_tensor_scan instruction
    (h_t = dA_t * h_{t-1} + dBx_t along the free/time axis).
  - out_proj partials reduce-scattered within the b-group back to
    channel tile q; each core emits its [128, 2048] output shard.

Everything channel-major [channel, time] on-chip; no transposes.
"""

import os
import sys
import numpy as np

for _p in ("/opt/trn_rl_repo", "/root/.axon_site/_ro/trn_rl_repo"):
    if os.path.isdir(_p) and _p not in sys.path:
        sys.path.append(_p)

import ml_dtypes  # noqa: E402

from concourse import bass, bacc, mybir, tile  # noqa: E402
from concourse.bass_utils import run_bass_kernel_spmd  # noqa: E402

F32 = mybir.dt.float32
BF16 = mybir.dt.bfloat16
AF = mybir.ActivationFunctionType
OP = mybir.AluOpType

B, L, C, DI, N, RK, KK, DC = 2, 2048, 512, 1024, 16, 32, 7, 4
S = DI // 4      # 256 d_inner channels per core
CT = C // 4      # 128 output channels per core
P = 128
LB = 512         # l-block (one PSUM bank of f32)
NLB = L // LB
EPS = 1e-5

BCAST_DMA = os.environ.get("BCAST_DMA", "0") == "1"

GRP_B = [[0, 1, 2, 3], [4, 5, 6, 7]]        # b-groups
GRP_PAIR = [[0, 4], [1, 5], [2, 6], [3, 7]]  # same-ctile pairs for BN stats


def build_graph():
    nc = bacc.Bacc("TRN2", target_bir_lowering=False, debug=False,
                   num_devices=8)

    # ---- kernel I/O --------------------------------------------------
    xb = nc.dram_tensor("xb", [C, L], BF16, kind="ExternalInput")
    xct = nc.dram_tensor("xct", [CT, L], F32, kind="ExternalInput")
    ffw = nc.dram_tensor("ffw", [KK, C, CT], BF16, kind="ExternalInput")
    ffb = nc.dram_tensor("ffb", [CT, 1], F32, kind="ExternalInput")
    gamma = nc.dram_tensor("gamma", [CT, 1], F32, kind="ExternalInput")
    beta = nc.dram_tensor("beta", [CT, 1], F32, kind="ExternalInput")
    winx = nc.dram_tensor("winx", [C, S], BF16, kind="ExternalInput")
    winz = nc.dram_tensor("winz", [C, S], BF16, kind="ExternalInput")
    convd = nc.dram_tensor("convd", [DC, S, P], BF16, kind="ExternalInput")
    convb = nc.dram_tensor("convb", [S, 1], F32, kind="ExternalInput")
    wxp = nc.dram_tensor("wxp", [S, RK + 2 * N], BF16, kind="ExternalInput")
    wdt = nc.dram_tensor("wdt", [RK, S], BF16, kind="ExternalInput")
    dtb = nc.dram_tensor("dtb", [S, 1], F32, kind="ExternalInput")
    acol = nc.dram_tensor("acol", [S, N], F32, kind="ExternalInput")
    dskip = nc.dram_tensor("dskip", [S, 1], F32, kind="ExternalInput")
    wout = nc.dram_tensor("wout", [S, C], BF16, kind="ExternalInput")
    ident = nc.dram_tensor("ident", [P, P], BF16, kind="ExternalInput")
    out = nc.dram_tensor("out", [CT, L], F32, kind="ExternalOutput")

    with tile.TileContext(nc) as tc:
        _emit(nc, tc, xb, xct, ffw, ffb, gamma, beta, winx, winz, convd,
              convb, wxp, wdt, dtb, acol, dskip, wout, ident, out)

    nc.compile()
    return nc


def _emit(nc, tc, xb, xct, ffw, ffb, gamma, beta, winx, winz, convd, convb,
          wxp, wdt, dtb, acol, dskip, wout, ident, out):
    sync = nc.sync
    vec = nc.vector
    act = nc.scalar
    pe = nc.tensor
    gps = nc.gpsimd

    import contextlib
    _pers_ctx = contextlib.ExitStack()
    _pers = _pers_ctx.enter_context(tc.tile_pool(name="pers", bufs=1))

    def stile(shape, dtype, name):
        return _pers.tile(shape, dtype, name=name, tag=name)

    # ---- DRAM bounce buffers for collectives -------------------------
    with tc.tile_pool(name="dram", bufs=1, space="DRAM") as dram:
        bn_in = dram.tile([CT, 2], F32, name="bn_in")
        bn_out = dram.tile([CT, 2], F32, name="bn_out")
        ng_in_q = [dram.tile([CT, LB], BF16, name=f"ng_in{j}")
                   for j in range(NLB)]
        ng_out_q = [dram.tile([C, LB], BF16, name=f"ng_out{j}")
                    for j in range(NLB)]
        dbc_in_h = [dram.tile([RK + 2 * N, L // 2], BF16, name=f"dbc_in{h}")
                    for h in range(2)]
        dbc_out_h = [dram.tile([RK + 2 * N, L // 2], BF16,
                               name=f"dbc_out{h}") for h in range(2)]
        att_in_q = [dram.tile([C, LB], BF16, name=f"att_in{j}")
                    for j in range(NLB)]
        att_out_q = [dram.tile([CT, LB], BF16, name=f"att_out{j}")
                     for j in range(NLB)]

        # ---- persistent SBUF tiles ----------------------------------
        ffb_sb = stile([CT, 1], F32, "ffb_sb")
        gamma_sb = stile([CT, 1], F32, "gamma_sb")
        beta_sb = stile([CT, 1], F32, "beta_sb")
        wxp_sb = stile([P, 2 * (RK + 2 * N)], BF16, "wxp_sb")
        wdt_sb = stile([RK, S], BF16, "wdt_sb")
        dtb_sb = stile([P, 2], F32, "dtb_sb")
        acol_sb = stile([P, 2 * N], F32, "acol_sb")
        dskip_sb = stile([P, 2], F32, "dskip_sb")
        convd_sb = stile([P, DC * 2 * P], BF16, "convd_sb")
        convb_sb = stile([P, 2], F32, "convb_sb")
        wout_sb = stile([P, 8 * P], BF16, "wout_sb")
        xct_sb = stile([CT, L], F32, "xct_sb")

        ff_out = stile([CT, L], F32, "ff_out")
        xs_act = [stile([P, L], F32, f"xs_act{d}") for d in range(2)]
        zsil = [stile([P, L], F32, f"zsil{d}") for d in range(2)]
        dtT = [stile([P, L], F32, f"dtT{d}") for d in range(2)]
        vT = [stile([P, L], BF16, f"vT{d}") for d in range(2)]
        idt_sb = stile([P, P], BF16, "idt_sb")
        yacc = [stile([P, L], F32, f"yacc{d}") for d in range(2)]

        sync.dma_start(out=ffb_sb[:], in_=ffb.ap()[:, :])

        # =============================================================
        # Phase 1: ff conv (Conv1d k=7 same-pad) + ReLU -> ff_out
        # =============================================================
        with tc.tile_pool(name="ffpool", bufs=1) as ffp, \
             tc.tile_pool(name="ffpsum", bufs=3, space="PSUM") as ffps:
            ffw_sb = ffp.tile([P, KK * 4 * P], BF16, name="ffw_sb")
            for ci in range(4):
                eng = (act, gps, sync, act)[ci]
                eng.dma_start(
                    out=ffw_sb[:, ci * KK * P:(ci + 1) * KK * P]
                    .rearrange("p (k m) -> p k m", k=KK),
                    in_=ffw.ap()[:, ci * P:(ci + 1) * P, :]
                    .rearrange("k p m -> p k m"))
            x_sb = []
            for ci in range(4):
                t = ffp.tile([P, L + 6], BF16, name=f"x_sb{ci}")
                gps.memset(t[:, 0:3], 0.0)
                gps.memset(t[:, L + 3:L + 6], 0.0)
                (sync, act, gps, sync)[ci].dma_start(
                    out=t[:, 3:L + 3], in_=xb.ap()[ci * P:(ci + 1) * P, :])
                x_sb.append(t)

            for lb in range(NLB):
                ps = ffps.tile([P, LB], F32, name="ffps")
                nmm = KK * 4
                j = 0
                for k in range(KK):
                    for ci in range(4):
                        jj = ci * KK + k
                        pe.matmul(
                            out=ps[:],
                            lhsT=ffw_sb[:, jj * P:(jj + 1) * P],
                            rhs=x_sb[ci][:, k + lb * LB:k + lb * LB + LB],
                            start=(j == 0), stop=(j == nmm - 1))
                        j += 1
                act.activation(out=ff_out[:, lb * LB:(lb + 1) * LB], in_=ps[:],
                               func=AF.Relu, bias=ffb_sb[:, 0:1])

        sync.dma_start(out=gamma_sb[:], in_=gamma.ap()[:, :])
        sync.dma_start(out=beta_sb[:], in_=beta.ap()[:, :])
        sync.dma_start(out=wdt_sb[:], in_=wdt.ap()[:, :])
        sync.dma_start(out=xct_sb[:], in_=xct.ap()[:, :])
        sync.dma_start(out=idt_sb[:], in_=ident.ap()[:, :])
        for d in range(2):
            rs = slice(d * P, (d + 1) * P)
            sync.dma_start(out=wxp_sb[:, d * 64:(d + 1) * 64],
                           in_=wxp.ap()[rs, :])
            sync.dma_start(out=dtb_sb[:, d:d + 1], in_=dtb.ap()[rs, :])
            sync.dma_start(out=acol_sb[:, d * N:(d + 1) * N],
                           in_=acol.ap()[rs, :])
            sync.dma_start(out=dskip_sb[:, d:d + 1], in_=dskip.ap()[rs, :])
            sync.dma_start(out=convb_sb[:, d:d + 1], in_=convb.ap()[rs, :])
            sync.dma_start(out=wout_sb[:, d * 4 * P:(d + 1) * 4 * P],
                           in_=wout.ap()[rs, :])
            sync.dma_start(
                out=convd_sb[:, d * DC * P:(d + 1) * DC * P]
                .rearrange("p (k m) -> p k m", k=DC),
                in_=convd.ap()[:, d * P:(d + 1) * P, :]
                .rearrange("k p m -> p k m"))

        # =============================================================
        # Phase 2: BN stats + pairwise AllReduce + normalize (-> bf16)
        # =============================================================
        with tc.tile_pool(name="bnpool", bufs=1) as bnp:
            stat = bnp.tile([CT, 2], F32, name="stat")
            stat2 = bnp.tile([CT, 2], F32, name="stat2")
            sq = bnp.tile([CT, L], BF16, name="sq")
            vec.tensor_reduce(out=stat[:, 0:1], in_=ff_out[:],
                              axis=mybir.AxisListType.X, op=OP.add)
            act.activation(out=sq[:], in_=ff_out[:], func=AF.Square,
                           accum_out=stat[:, 1:2])
            sync.dma_start(out=bn_in[:], in_=stat[:])
            gps.collective_compute("AllReduce", OP.add,
                                   replica_groups=GRP_PAIR,
                                   ins=[bn_in.opt()], outs=[bn_out.opt()])
            sync.dma_start(out=stat2[:], in_=bn_out[:])

            mu = bnp.tile([CT, 1], F32, name="mu")
            ex2 = bnp.tile([CT, 1], F32, name="ex2")
            msq = bnp.tile([CT, 1], F32, name="msq")
            var = bnp.tile([CT, 1], F32, name="var")
            std = bnp.tile([CT, 1], F32, name="std")
            rstd = bnp.tile([CT, 1], F32, name="rstd")
            bscale = bnp.tile([CT, 1], F32, name="bscale")
            tmp1 = bnp.tile([CT, 1], F32, name="tmp1")
            bshift = bnp.tile([CT, 1], F32, name="bshift")
            act.mul(mu[:], stat2[:, 0:1], 1.0 / (B * L))
            act.mul(ex2[:], stat2[:, 1:2], 1.0 / (B * L))
            act.square(msq[:], mu[:])
            vec.tensor_tensor(out=var[:], in0=ex2[:], in1=msq[:],
                              op=OP.subtract)
            vec.tensor_scalar_add(out=var[:], in0=var[:], scalar1=EPS)
            act.activation(out=std[:], in_=var[:], func=AF.Sqrt)
            vec.reciprocal(rstd[:], std[:])
            vec.tensor_tensor(out=bscale[:], in0=rstd[:], in1=gamma_sb[:],
                              op=OP.mult)
            vec.tensor_tensor(out=tmp1[:], in0=mu[:], in1=bscale[:],
                              op=OP.mult)
            vec.tensor_tensor(out=bshift[:], in0=beta_sb[:], in1=tmp1[:],
                              op=OP.subtract)

            nrm_t = bnp.tile([CT, L], F32, name="nrm_t")
            nrm_l = bnp.tile([CT, L], BF16, name="nrm_l")
            act.activation(out=nrm_t[:], in_=ff_out[:], func=AF.Copy,
                           scale=bscale[:, 0:1])
            vec.tensor_scalar_add(out=nrm_l[:], in0=nrm_t[:],
                                  scalar1=bshift[:, 0:1])
            for j in range(NLB):
                sync.dma_start(out=ng_in_q[j][:],
                               in_=nrm_l[:, j * LB:(j + 1) * LB])

        # =============================================================
        # Phase 3: AllGather normed within b-group
        # =============================================================
        for j in range(NLB):
            gps.collective_compute("AllGather", OP.bypass,
                                   replica_groups=GRP_B,
                                   ins=[ng_in_q[j].opt()],
                                   outs=[ng_out_q[j].opt()])
        H = L // 2

        # =============================================================
        # Phase 4: in_proj (xs & z), depthwise conv, silu
        # =============================================================
        with tc.tile_pool(name="ippool", bufs=1) as ipp, \
             tc.tile_pool(name="ippsum", bufs=2, space="PSUM") as ipps, \
             tc.tile_pool(name="xsppool", bufs=1) as xspp:
            nrm_q = []
            for j in range(NLB):
                row_t = []
                for ci in range(4):
                    t = ipp.tile([P, LB], BF16, name=f"nrm{j}_{ci}")
                    sync.dma_start(out=t[:],
                                   in_=ng_out_q[j][ci * P:(ci + 1) * P, :])
                    row_t.append(t)
                nrm_q.append(row_t)
            winx_sb = ipp.tile([P, 8 * P], BF16, name="winx_sb")
            winz_sb = ipp.tile([P, 8 * P], BF16, name="winz_sb")
            for ci in range(4):
                sync.dma_start(out=winx_sb[:, ci * 2 * P:(ci + 1) * 2 * P],
                               in_=winx.ap()[ci * P:(ci + 1) * P, :])
                sync.dma_start(out=winz_sb[:, ci * 2 * P:(ci + 1) * 2 * P],
                               in_=winz.ap()[ci * P:(ci + 1) * P, :])

            xsp = [xspp.tile([P, L + 3], BF16, name=f"xsp{d}")
                   for d in range(2)]
            for d in range(2):
                gps.memset(xsp[d][:, 0:3], 0.0)

            for lb in range(NLB):
                hh, loc = lb, 0
                for d in range(2):
                    ps = ipps.tile([P, LB], F32, name="xzps")
                    for ci in range(4):
                        pe.matmul(out=ps[:],
                                  lhsT=winx_sb[:, (ci * 2 + d) * P:(ci * 2 + d + 1) * P],
                                  rhs=nrm_q[hh][ci][:, loc:loc + LB],
                                  start=(ci == 0), stop=(ci == 3))
                    act.copy(xsp[d][:, 3 + lb * LB:3 + (lb + 1) * LB], ps[:])
                    ps2 = ipps.tile([P, LB], F32, name="zps")
                    for ci in range(4):
                        pe.matmul(out=ps2[:],
                                  lhsT=winz_sb[:, (ci * 2 + d) * P:(ci * 2 + d + 1) * P],
                                  rhs=nrm_q[hh][ci][:, loc:loc + LB],
                                  start=(ci == 0), stop=(ci == 3))
                    act.activation(out=zsil[d][:, lb * LB:(lb + 1) * LB],
                                   in_=ps2[:], func=AF.Silu)

            # depthwise causal conv: 4 diagonal matmuls per (d, lb)
            with tc.tile_pool(name="cvpsum", bufs=3, space="PSUM") as cvps:
                for d in range(2):
                    for lb in range(NLB):
                        ps3 = cvps.tile([P, LB], F32, name="cvps")
                        for k in range(DC):
                            jj = d * DC + k
                            pe.matmul(
                                out=ps3[:],
                                lhsT=convd_sb[:, jj * P:(jj + 1) * P],
                                rhs=xsp[d][:, k + lb * LB:k + lb * LB + LB],
                                start=(k == 0), stop=(k == DC - 1))
                        act.activation(out=xs_act[d][:, lb * LB:(lb + 1) * LB],
                                       in_=ps3[:], func=AF.Silu,
                                       bias=convb_sb[:, d:d + 1])

        # =============================================================
        # Phase 5+6 (per L-half, overlapping the other half\'s scan):
        # x_proj partial + AllReduce -> dt_raw/Bm/Cm; dt = softplus; v
        # =============================================================
        xpp_ctx = contextlib.ExitStack()
        xpp = xpp_ctx.enter_context(tc.tile_pool(name="xppool", bufs=1))
        xpps = xpp_ctx.enter_context(
            tc.tile_pool(name="xppsum", bufs=1, space="PSUM"))
        dtps = xpp_ctx.enter_context(
            tc.tile_pool(name="dtpsum", bufs=1, space="PSUM"))
        xs_b16 = [xpp.tile([P, L], BF16, name=f"xs_b16{d}")
                  for d in range(2)]
        for d in range(2):
            act.copy(xs_b16[d][:], xs_act[d][:])
        for half in range(2):
            o = half * H
            dbc_sb = xpp.tile([RK + 2 * N, H], BF16, name="dbc_sb", bufs=2)
            for j in range(2):
                ps = xpps.tile([RK + 2 * N, LB], F32, name="dbcps")
                for d in range(2):
                    pe.matmul(out=ps[:],
                              lhsT=wxp_sb[:, d * 64:(d + 1) * 64],
                              rhs=xs_b16[d][:, o + j * LB:o + (j + 1) * LB],
                              start=(d == 0), stop=(d == 1))
                act.copy(dbc_sb[:, j * LB:(j + 1) * LB], ps[:])
            sync.dma_start(out=dbc_in_h[half][:], in_=dbc_sb[:])
            gps.collective_compute("AllReduce", OP.add,
                                   replica_groups=GRP_B,
                                   ins=[dbc_in_h[half].opt()],
                                   outs=[dbc_out_h[half].opt()])
            dtr = xpp.tile([RK, H], BF16, name="dtr", bufs=2)
            sync.dma_start(out=dtr[:], in_=dbc_out_h[half][0:RK, :])
            for d in range(2):
                for j in range(2):
                    ps = dtps.tile([P, LB], F32, name="dtps")
                    pe.matmul(out=ps[:],
                              lhsT=wdt_sb[:, d * P:(d + 1) * P],
                              rhs=dtr[:, j * LB:(j + 1) * LB],
                              start=True, stop=True)
                    # softplus(x) = ln(1 + exp(x)) (no softplus ACT table)
                    et = xpp.tile([P, LB], F32, name="et", bufs=2)
                    act.activation(out=et[:], in_=ps[:], func=AF.Exp,
                                   bias=dtb_sb[:, d:d + 1])
                    act.activation(
                        out=dtT[d][:, o + j * LB:o + (j + 1) * LB],
                        in_=et[:], func=AF.Ln, bias=1.0)
                vec.tensor_tensor(out=vT[d][:, o:o + H],
                                  in0=dtT[d][:, o:o + H],
                                  in1=xs_act[d][:, o:o + H], op=OP.mult)

        # =============================================================
        # Phase 7-9, pipelined over L-halves:
        #   per half: per (n, d): dA = exp(A[:,n]*dt); dBx = v*Bm_n;
        #   h = scan(dA, dBx) [state handoff between halves];
        #   prod = h*Cm_n; PE identity-matmul accumulates sum_n in PSUM.
        #   Then gate + out_proj + ReduceScatter + residual for the half,
        #   overlapping the other half\'s scan on the vector engine.
        # =============================================================
        with tc.tile_pool(name="bmb", bufs=3) as bmbp, \
             tc.tile_pool(name="cmb", bufs=3) as cmbp, \
             tc.tile_pool(name="sca", bufs=2) as scap, \
             tc.tile_pool(name="scb", bufs=3) as scbp, \
             tc.tile_pool(name="sch", bufs=2) as schp, \
             tc.tile_pool(name="ygpool", bufs=1) as ygp, \
             tc.tile_pool(name="fin", bufs=1) as finp, \
             tc.tile_pool(name="ypsum", bufs=1, space="PSUM") as ypsp, \
             tc.tile_pool(name="atpsum", bufs=2, space="PSUM") as atps:
            hfin = stile([P, 2 * N], F32, "hfin")
            yg = [ygp.tile([P, L], BF16, name=f"yg{d}") for d in range(2)]
            att_sb = finp.tile([CT, L], BF16, name="att_sb")
            out_sb = finp.tile([CT, L], F32, name="out_sb")
            for q in range(NLB):
                o = q * LB
                yps = [ypsp.tile([P, LB], F32, name=f"yps{d}",
                                 tag=f"yps{d}") for d in range(2)]
                for n in range(N):
                    bc = bmbp.tile([P, 2 * LB], BF16, name="bc")
                    brow = cmbp.tile([1, 2 * LB], BF16, name="brow", bufs=2)
                    hh, ho = q // 2, (q % 2) * LB
                    sync.dma_start(
                        out=brow[:, 0:LB],
                        in_=dbc_out_h[hh][RK + n:RK + n + 1, ho:ho + LB])
                    sync.dma_start(
                        out=brow[:, LB:2 * LB],
                        in_=dbc_out_h[hh][RK + N + n:RK + N + n + 1,
                                          ho:ho + LB])
                    gps.partition_broadcast(bc[:], brow[:])
                    bmb = bc[:, 0:LB]
                    cmb = bc[:, LB:2 * LB]
                    for d in range(2):
                        idx = n * 2 + d
                        da = scap.tile([P, LB], BF16, name="da")
                        dbx = scbp.tile([P, LB], BF16, name="dbx")
                        hs = schp.tile([P, LB], BF16, name="hs")
                        act.activation(
                            out=da[:], in_=dtT[d][:, o:o + LB], func=AF.Exp,
                            scale=acol_sb[:, d * N + n:d * N + n + 1])
                        vec.tensor_tensor(out=dbx[:], in0=vT[d][:, o:o + LB],
                                          in1=bmb, op=OP.mult)
                        vec.tensor_tensor_scan(
                            out=hs[:], data0=da[:], data1=dbx[:],
                            initial=(0.0 if q == 0
                                     else hfin[:, idx:idx + 1]),
                            op0=OP.mult, op1=OP.add)
                        if q < NLB - 1:
                            act.copy(hfin[:, idx:idx + 1], hs[:, LB - 1:LB])
                        vec.tensor_tensor(out=dbx[:], in0=hs[:], in1=cmb,
                                          op=OP.mult)
                        pe.matmul(out=yps[d][:], lhsT=idt_sb[:],
                                  rhs=dbx[:],
                                  start=(n == 0), stop=(n == N - 1),
                                  skip_group_check=True)

                # ---- evacuate + gate + out_proj + RS for this quarter ----
                for d in range(2):
                    act.copy(yacc[d][:, o:o + LB], yps[d][:])
                    vec.scalar_tensor_tensor(
                        out=yacc[d][:, o:o + LB], in0=xs_act[d][:, o:o + LB],
                        scalar=dskip_sb[:, d:d + 1], in1=yacc[d][:, o:o + LB],
                        op0=OP.mult, op1=OP.add)
                    vec.tensor_tensor(out=yg[d][:, o:o + LB],
                                      in0=yacc[d][:, o:o + LB],
                                      in1=zsil[d][:, o:o + LB], op=OP.mult)
                for ct in range(4):
                    ps = atps.tile([P, LB], F32, name="atps")
                    for d in range(2):
                        pe.matmul(
                            out=ps[:],
                            lhsT=wout_sb[:, (d * 4 + ct) * P:(d * 4 + ct + 1) * P],
                            rhs=yg[d][:, o:o + LB],
                            start=(d == 0), stop=(d == 1))
                    st = ygp.tile([P, LB], BF16, name="atstage", bufs=4)
                    act.copy(st[:], ps[:])
                    sync.dma_start(out=att_in_q[q][ct * P:(ct + 1) * P, :],
                                   in_=st[:])
                gps.collective_compute("ReduceScatter", OP.add,
                                       replica_groups=GRP_B,
                                       ins=[att_in_q[q].opt()],
                                       outs=[att_out_q[q].opt()])
                sync.dma_start(out=att_sb[:, o:o + LB], in_=att_out_q[q][:])
                vec.tensor_tensor(out=out_sb[:, o:o + LB],
                                  in0=att_sb[:, o:o + LB],
                                  in1=ff_out[:, o:o + LB], op=OP.add)
                vec.tensor_tensor(out=out_sb[:, o:o + LB],
                                  in0=out_sb[:, o:o + LB],
                                  in1=xct_sb[:, o:o + LB], op=OP.add)
                sync.dma_start(out=out.ap()[:, o:o + LB],
                               in_=out_sb[:, o:o + LB])

        xpp_ctx.close()
    _pers_ctx.close()


_NC_CACHE = None
LAST_EXEC_NS = None


def _get_nc():
    global _NC_CACHE
    if _NC_CACHE is None:
        _NC_CACHE = build_graph()
    return _NC_CACHE


def make_in_maps(inputs):
    f32 = lambda a: np.ascontiguousarray(np.asarray(a), dtype=np.float32)
    bf16 = lambda a: np.ascontiguousarray(
        np.asarray(a, dtype=np.float32).astype(ml_dtypes.bfloat16))
    x = f32(inputs["x"])
    ff_w = f32(inputs["ff_w"])
    ff_b = f32(inputs["ff_b"])
    g = f32(inputs["bn_gamma"])
    bt = f32(inputs["bn_beta"])
    w_in = f32(inputs["w_in"])
    conv_w = f32(inputs["conv_w"])
    conv_b = f32(inputs["conv_b"])
    w_xproj = f32(inputs["w_xproj"])
    w_dt = f32(inputs["w_dt"])
    dt_bias = f32(inputs["dt_bias"])
    A = -np.exp(f32(inputs["A_log"]))
    D_skip = f32(inputs["D_skip"])
    w_out = f32(inputs["w_out"])
    ffw_t = np.transpose(ff_w, (2, 1, 0))  # [K, C, co]

    in_maps = []
    for i in range(8):
        b, q = i // 4, i % 4
        dsl = slice(q * S, (q + 1) * S)
        csl = slice(q * CT, (q + 1) * CT)
        in_maps.append({
            "xb": bf16(x[b]),
            "xct": f32(x[b, csl]),
            "ffw": bf16(ffw_t[:, :, csl]),
            "ffb": f32(ff_b[csl].reshape(CT, 1)),
            "gamma": f32(g[csl].reshape(CT, 1)),
            "beta": f32(bt[csl].reshape(CT, 1)),
            "winx": bf16(w_in[:, :DI][:, dsl]),
            "winz": bf16(w_in[:, DI:][:, dsl]),
            "convd": bf16(np.stack([
                np.stack([np.diag(conv_w[dsl][dd * P:(dd + 1) * P, k])
                          for dd in range(2)]).reshape(S, P)
                for k in range(DC)])),
            "convb": f32(conv_b[dsl].reshape(S, 1)),
            "wxp": bf16(w_xproj[dsl]),
            "wdt": bf16(w_dt[:, dsl]),
            "dtb": f32(dt_bias[dsl].reshape(S, 1)),
            "acol": f32(A[dsl]),
            "dskip": f32(D_skip[dsl].reshape(S, 1)),
            "wout": bf16(w_out[dsl]),
            "ident": np.eye(P, dtype=np.float32).astype(ml_dtypes.bfloat16),
        })
    return in_maps


def _install_ntff_hook():
    """The agent image's antenv lacks axon_hooks; recreate it so
    run_bass_kernel_spmd(trace=True) can NTFF-profile via the axon .so."""
    import types
    if "antenv.axon_hooks" in sys.modules:
        return
    try:
        from trn_agent_boot.trn_boot import _ntff_profile_via_ctypes
        hook = _ntff_profile_via_ctypes("/opt/axon/libaxon_pjrt.so")
    except Exception:
        hook = None
    mod = types.ModuleType("antenv.axon_hooks")
    mod.get_axon_ntff_profile_hook = lambda: hook
    mod.set_axon_ntff_profile_hook = lambda h: None
    sys.modules["antenv.axon_hooks"] = mod


def kernel(**inputs):
    global LAST_EXEC_NS
    nc = _get_nc()
    in_maps = make_in_maps(inputs)
    trace = os.environ.get("KERNEL_TRACE", "0") == "1"
    if trace:
        _install_ntff_hook()
    try:
        res = run_bass_kernel_spmd(nc, in_maps, core_ids=list(range(8)),
                                   trace=trace)
    except Exception:
        if not trace:
            raise
        res = run_bass_kernel_spmd(nc, in_maps, core_ids=list(range(8)),
                                   trace=False)
    LAST_EXEC_NS = res.exec_time_ns
    out = np.empty((B, C, L), dtype=np.float32)
    for i in range(8):
        b, q = i // 4, i % 4
        out[b, q * CT:(q + 1) * CT] = res.results[i]["out"]
    return out


# revision 34
# speedup vs baseline: 1.1461x; 1.1461x over previous
"""Trainium2 8-core kernel for the ConvFF + BatchNorm + Mamba block.

Sharding (8 NeuronCores): core i -> b = i//4 (batch), q = i%4.
  - Front (ff conv + BN): computes output-channel tile q (128 of 512
    channels) for batch b. BN stats all-reduced across the b-pair.
  - normed all-gathered within each b-group of 4 cores.
  - Mamba: d_inner slice q (256 of 1024 channels) for batch b; the
    x_proj partial is all-reduced within the b-group; the selective
    scan runs fully local via the DVE tensor_tensor_scan instruction
    (h_t = dA_t * h_{t-1} + dBx_t along the free/time axis).
  - out_proj partials reduce-scattered within the b-group back to
    channel tile q; each core emits its [128, 2048] output shard.

Everything channel-major [channel, time] on-chip; no transposes.
"""

import os
import sys
import numpy as np

for _p in ("/opt/trn_rl_repo", "/root/.axon_site/_ro/trn_rl_repo"):
    if os.path.isdir(_p) and _p not in sys.path:
        sys.path.append(_p)

import ml_dtypes  # noqa: E402

from concourse import bass, bacc, mybir, tile  # noqa: E402
from concourse.bass_utils import run_bass_kernel_spmd  # noqa: E402

F32 = mybir.dt.float32
BF16 = mybir.dt.bfloat16
AF = mybir.ActivationFunctionType
OP = mybir.AluOpType

B, L, C, DI, N, RK, KK, DC = 2, 2048, 512, 1024, 16, 32, 7, 4
S = DI // 4      # 256 d_inner channels per core
CT = C // 4      # 128 output channels per core
P = 128
LB = 512         # l-block (one PSUM bank of f32)
NLB = L // LB
EPS = 1e-5

BCAST_DMA = os.environ.get("BCAST_DMA", "0") == "1"

GRP_B = [[0, 1, 2, 3], [4, 5, 6, 7]]        # b-groups
GRP_PAIR = [[0, 4], [1, 5], [2, 6], [3, 7]]  # same-ctile pairs for BN stats


def build_graph():
    nc = bacc.Bacc("TRN2", target_bir_lowering=False, debug=False,
                   num_devices=8)

    # ---- kernel I/O --------------------------------------------------
    xb = nc.dram_tensor("xb", [C, L], BF16, kind="ExternalInput")
    xct = nc.dram_tensor("xct", [CT, L], F32, kind="ExternalInput")
    ffw = nc.dram_tensor("ffw", [KK, C, CT], BF16, kind="ExternalInput")
    ffb = nc.dram_tensor("ffb", [CT, 1], F32, kind="ExternalInput")
    gamma = nc.dram_tensor("gamma", [CT, 1], F32, kind="ExternalInput")
    beta = nc.dram_tensor("beta", [CT, 1], F32, kind="ExternalInput")
    winx = nc.dram_tensor("winx", [C, S], BF16, kind="ExternalInput")
    winz = nc.dram_tensor("winz", [C, S], BF16, kind="ExternalInput")
    convd = nc.dram_tensor("convd", [DC, S, P], BF16, kind="ExternalInput")
    convb = nc.dram_tensor("convb", [S, 1], F32, kind="ExternalInput")
    wxp = nc.dram_tensor("wxp", [S, RK + 2 * N], BF16, kind="ExternalInput")
    wdt = nc.dram_tensor("wdt", [RK, S], BF16, kind="ExternalInput")
    dtb = nc.dram_tensor("dtb", [S, 1], F32, kind="ExternalInput")
    acol = nc.dram_tensor("acol", [S, N], F32, kind="ExternalInput")
    dskip = nc.dram_tensor("dskip", [S, 1], F32, kind="ExternalInput")
    wout = nc.dram_tensor("wout", [S, C], BF16, kind="ExternalInput")
    ident = nc.dram_tensor("ident", [P, P], BF16, kind="ExternalInput")
    out = nc.dram_tensor("out", [CT, L], F32, kind="ExternalOutput")

    with tile.TileContext(nc) as tc:
        _emit(nc, tc, xb, xct, ffw, ffb, gamma, beta, winx, winz, convd,
              convb, wxp, wdt, dtb, acol, dskip, wout, ident, out)

    nc.compile()
    return nc


def _emit(nc, tc, xb, xct, ffw, ffb, gamma, beta, winx, winz, convd, convb,
          wxp, wdt, dtb, acol, dskip, wout, ident, out):
    sync = nc.sync
    vec = nc.vector
    act = nc.scalar
    pe = nc.tensor
    gps = nc.gpsimd

    import contextlib
    _pers_ctx = contextlib.ExitStack()
    _pers = _pers_ctx.enter_context(tc.tile_pool(name="pers", bufs=1))

    def stile(shape, dtype, name):
        return _pers.tile(shape, dtype, name=name, tag=name)

    # ---- DRAM bounce buffers for collectives -------------------------
    with tc.tile_pool(name="dram", bufs=1, space="DRAM") as dram:
        bn_in = dram.tile([CT, 2], F32, name="bn_in")
        bn_out = dram.tile([CT, 2], F32, name="bn_out")
        ng_in0 = dram.tile([CT, L // 2], BF16, name="ng_in0")
        ng_in1 = dram.tile([CT, L // 2], BF16, name="ng_in1")
        ng_out0 = dram.tile([C, L // 2], BF16, name="ng_out0")
        ng_out1 = dram.tile([C, L // 2], BF16, name="ng_out1")
        dbc_in_h = [dram.tile([RK + 2 * N, L // 2], BF16, name=f"dbc_in{h}")
                    for h in range(2)]
        dbc_out_h = [dram.tile([RK + 2 * N, L // 2], BF16,
                               name=f"dbc_out{h}") for h in range(2)]
        att_in0 = dram.tile([C, L // 2], BF16, name="att_in0")
        att_in1 = dram.tile([C, L // 2], BF16, name="att_in1")
        att_out0 = dram.tile([CT, L // 2], BF16, name="att_out0")
        att_out1 = dram.tile([CT, L // 2], BF16, name="att_out1")

        # ---- persistent SBUF tiles ----------------------------------
        ffb_sb = stile([CT, 1], F32, "ffb_sb")
        gamma_sb = stile([CT, 1], F32, "gamma_sb")
        beta_sb = stile([CT, 1], F32, "beta_sb")
        wxp_sb = stile([P, 2 * (RK + 2 * N)], BF16, "wxp_sb")
        wdt_sb = stile([RK, S], BF16, "wdt_sb")
        dtb_sb = stile([P, 2], F32, "dtb_sb")
        acol_sb = stile([P, 2 * N], F32, "acol_sb")
        dskip_sb = stile([P, 2], F32, "dskip_sb")
        convd_sb = stile([P, DC * 2 * P], BF16, "convd_sb")
        convb_sb = stile([P, 2], F32, "convb_sb")
        wout_sb = stile([P, 8 * P], BF16, "wout_sb")
        xct_sb = stile([CT, L], F32, "xct_sb")

        ff_out = stile([CT, L], F32, "ff_out")
        xs_act = [stile([P, L], F32, f"xs_act{d}") for d in range(2)]
        zsil = [stile([P, L], F32, f"zsil{d}") for d in range(2)]
        dtT = [stile([P, L], F32, f"dtT{d}") for d in range(2)]
        vT = [stile([P, L], BF16, f"vT{d}") for d in range(2)]
        idt_sb = stile([P, P], BF16, "idt_sb")
        yacc = [stile([P, L], F32, f"yacc{d}") for d in range(2)]

        sync.dma_start(out=ffb_sb[:], in_=ffb.ap()[:, :])

        # =============================================================
        # Phase 1: ff conv (Conv1d k=7 same-pad) + ReLU -> ff_out
        # =============================================================
        with tc.tile_pool(name="ffpool", bufs=1) as ffp, \
             tc.tile_pool(name="ffpsum", bufs=3, space="PSUM") as ffps:
            ffw_sb = ffp.tile([P, KK * 4 * P], BF16, name="ffw_sb")
            for ci in range(4):
                eng = (act, gps, sync, act)[ci]
                eng.dma_start(
                    out=ffw_sb[:, ci * KK * P:(ci + 1) * KK * P]
                    .rearrange("p (k m) -> p k m", k=KK),
                    in_=ffw.ap()[:, ci * P:(ci + 1) * P, :]
                    .rearrange("k p m -> p k m"))
            x_sb = []
            for ci in range(4):
                t = ffp.tile([P, L + 6], BF16, name=f"x_sb{ci}")
                gps.memset(t[:, 0:3], 0.0)
                gps.memset(t[:, L + 3:L + 6], 0.0)
                (sync, act, gps, sync)[ci].dma_start(
                    out=t[:, 3:L + 3], in_=xb.ap()[ci * P:(ci + 1) * P, :])
                x_sb.append(t)

            for lb in range(NLB):
                ps = ffps.tile([P, LB], F32, name="ffps")
                nmm = KK * 4
                j = 0
                for k in range(KK):
                    for ci in range(4):
                        jj = ci * KK + k
                        pe.matmul(
                            out=ps[:],
                            lhsT=ffw_sb[:, jj * P:(jj + 1) * P],
                            rhs=x_sb[ci][:, k + lb * LB:k + lb * LB + LB],
                            start=(j == 0), stop=(j == nmm - 1))
                        j += 1
                act.activation(out=ff_out[:, lb * LB:(lb + 1) * LB], in_=ps[:],
                               func=AF.Relu, bias=ffb_sb[:, 0:1])

        sync.dma_start(out=gamma_sb[:], in_=gamma.ap()[:, :])
        sync.dma_start(out=beta_sb[:], in_=beta.ap()[:, :])
        sync.dma_start(out=wdt_sb[:], in_=wdt.ap()[:, :])
        sync.dma_start(out=xct_sb[:], in_=xct.ap()[:, :])
        sync.dma_start(out=idt_sb[:], in_=ident.ap()[:, :])
        for d in range(2):
            rs = slice(d * P, (d + 1) * P)
            sync.dma_start(out=wxp_sb[:, d * 64:(d + 1) * 64],
                           in_=wxp.ap()[rs, :])
            sync.dma_start(out=dtb_sb[:, d:d + 1], in_=dtb.ap()[rs, :])
            sync.dma_start(out=acol_sb[:, d * N:(d + 1) * N],
                           in_=acol.ap()[rs, :])
            sync.dma_start(out=dskip_sb[:, d:d + 1], in_=dskip.ap()[rs, :])
            sync.dma_start(out=convb_sb[:, d:d + 1], in_=convb.ap()[rs, :])
            sync.dma_start(out=wout_sb[:, d * 4 * P:(d + 1) * 4 * P],
                           in_=wout.ap()[rs, :])
            sync.dma_start(
                out=convd_sb[:, d * DC * P:(d + 1) * DC * P]
                .rearrange("p (k m) -> p k m", k=DC),
                in_=convd.ap()[:, d * P:(d + 1) * P, :]
                .rearrange("k p m -> p k m"))

        # =============================================================
        # Phase 2: BN stats + pairwise AllReduce + normalize (-> bf16)
        # =============================================================
        with tc.tile_pool(name="bnpool", bufs=1) as bnp:
            stat = bnp.tile([CT, 2], F32, name="stat")
            stat2 = bnp.tile([CT, 2], F32, name="stat2")
            sq = bnp.tile([CT, L], BF16, name="sq")
            vec.tensor_reduce(out=stat[:, 0:1], in_=ff_out[:],
                              axis=mybir.AxisListType.X, op=OP.add)
            act.activation(out=sq[:], in_=ff_out[:], func=AF.Square,
                           accum_out=stat[:, 1:2])
            sync.dma_start(out=bn_in[:], in_=stat[:])
            gps.collective_compute("AllReduce", OP.add,
                                   replica_groups=GRP_PAIR,
                                   ins=[bn_in.opt()], outs=[bn_out.opt()])
            sync.dma_start(out=stat2[:], in_=bn_out[:])

            mu = bnp.tile([CT, 1], F32, name="mu")
            ex2 = bnp.tile([CT, 1], F32, name="ex2")
            msq = bnp.tile([CT, 1], F32, name="msq")
            var = bnp.tile([CT, 1], F32, name="var")
            std = bnp.tile([CT, 1], F32, name="std")
            rstd = bnp.tile([CT, 1], F32, name="rstd")
            bscale = bnp.tile([CT, 1], F32, name="bscale")
            tmp1 = bnp.tile([CT, 1], F32, name="tmp1")
            bshift = bnp.tile([CT, 1], F32, name="bshift")
            act.mul(mu[:], stat2[:, 0:1], 1.0 / (B * L))
            act.mul(ex2[:], stat2[:, 1:2], 1.0 / (B * L))
            act.square(msq[:], mu[:])
            vec.tensor_tensor(out=var[:], in0=ex2[:], in1=msq[:],
                              op=OP.subtract)
            vec.tensor_scalar_add(out=var[:], in0=var[:], scalar1=EPS)
            act.activation(out=std[:], in_=var[:], func=AF.Sqrt)
            vec.reciprocal(rstd[:], std[:])
            vec.tensor_tensor(out=bscale[:], in0=rstd[:], in1=gamma_sb[:],
                              op=OP.mult)
            vec.tensor_tensor(out=tmp1[:], in0=mu[:], in1=bscale[:],
                              op=OP.mult)
            vec.tensor_tensor(out=bshift[:], in0=beta_sb[:], in1=tmp1[:],
                              op=OP.subtract)

            nrm_t = bnp.tile([CT, L], F32, name="nrm_t")
            nrm_l = bnp.tile([CT, L], BF16, name="nrm_l")
            act.activation(out=nrm_t[:], in_=ff_out[:], func=AF.Copy,
                           scale=bscale[:, 0:1])
            vec.tensor_scalar_add(out=nrm_l[:], in0=nrm_t[:],
                                  scalar1=bshift[:, 0:1])
            sync.dma_start(out=ng_in0[:], in_=nrm_l[:, 0:L // 2])
            sync.dma_start(out=ng_in1[:], in_=nrm_l[:, L // 2:L])

        # =============================================================
        # Phase 3: AllGather normed within b-group
        # =============================================================
        gps.collective_compute("AllGather", OP.bypass,
                               replica_groups=GRP_B,
                               ins=[ng_in0.opt()], outs=[ng_out0.opt()])
        gps.collective_compute("AllGather", OP.bypass,
                               replica_groups=GRP_B,
                               ins=[ng_in1.opt()], outs=[ng_out1.opt()])
        H = L // 2

        # =============================================================
        # Phase 4: in_proj (xs & z), depthwise conv, silu
        # =============================================================
        with tc.tile_pool(name="ippool", bufs=1) as ipp, \
             tc.tile_pool(name="ippsum", bufs=2, space="PSUM") as ipps, \
             tc.tile_pool(name="xsppool", bufs=1) as xspp:
            nrm_h = [[], []]
            for h, ngo in ((0, ng_out0), (1, ng_out1)):
                for ci in range(4):
                    t = ipp.tile([P, H], BF16, name=f"nrm{h}_{ci}")
                    sync.dma_start(out=t[:], in_=ngo[ci * P:(ci + 1) * P, :])
                    nrm_h[h].append(t)
            winx_sb = ipp.tile([P, 8 * P], BF16, name="winx_sb")
            winz_sb = ipp.tile([P, 8 * P], BF16, name="winz_sb")
            for ci in range(4):
                sync.dma_start(out=winx_sb[:, ci * 2 * P:(ci + 1) * 2 * P],
                               in_=winx.ap()[ci * P:(ci + 1) * P, :])
                sync.dma_start(out=winz_sb[:, ci * 2 * P:(ci + 1) * 2 * P],
                               in_=winz.ap()[ci * P:(ci + 1) * P, :])

            xsp = [xspp.tile([P, L + 3], BF16, name=f"xsp{d}")
                   for d in range(2)]
            for d in range(2):
                gps.memset(xsp[d][:, 0:3], 0.0)

            for lb in range(NLB):
                hh, loc = lb // 2, (lb % 2) * LB
                for d in range(2):
                    ps = ipps.tile([P, LB], F32, name="xzps")
                    for ci in range(4):
                        pe.matmul(out=ps[:],
                                  lhsT=winx_sb[:, (ci * 2 + d) * P:(ci * 2 + d + 1) * P],
                                  rhs=nrm_h[hh][ci][:, loc:loc + LB],
                                  start=(ci == 0), stop=(ci == 3))
                    act.copy(xsp[d][:, 3 + lb * LB:3 + (lb + 1) * LB], ps[:])
                    ps2 = ipps.tile([P, LB], F32, name="zps")
                    for ci in range(4):
                        pe.matmul(out=ps2[:],
                                  lhsT=winz_sb[:, (ci * 2 + d) * P:(ci * 2 + d + 1) * P],
                                  rhs=nrm_h[hh][ci][:, loc:loc + LB],
                                  start=(ci == 0), stop=(ci == 3))
                    act.activation(out=zsil[d][:, lb * LB:(lb + 1) * LB],
                                   in_=ps2[:], func=AF.Silu)

            # depthwise causal conv: 4 diagonal matmuls per (d, lb)
            with tc.tile_pool(name="cvpsum", bufs=3, space="PSUM") as cvps:
                for d in range(2):
                    for lb in range(NLB):
                        ps3 = cvps.tile([P, LB], F32, name="cvps")
                        for k in range(DC):
                            jj = d * DC + k
                            pe.matmul(
                                out=ps3[:],
                                lhsT=convd_sb[:, jj * P:(jj + 1) * P],
                                rhs=xsp[d][:, k + lb * LB:k + lb * LB + LB],
                                start=(k == 0), stop=(k == DC - 1))
                        act.activation(out=xs_act[d][:, lb * LB:(lb + 1) * LB],
                                       in_=ps3[:], func=AF.Silu,
                                       bias=convb_sb[:, d:d + 1])

        # =============================================================
        # Phase 5+6 (per L-half, overlapping the other half\'s scan):
        # x_proj partial + AllReduce -> dt_raw/Bm/Cm; dt = softplus; v
        # =============================================================
        xpp_ctx = contextlib.ExitStack()
        xpp = xpp_ctx.enter_context(tc.tile_pool(name="xppool", bufs=1))
        xpps = xpp_ctx.enter_context(
            tc.tile_pool(name="xppsum", bufs=1, space="PSUM"))
        dtps = xpp_ctx.enter_context(
            tc.tile_pool(name="dtpsum", bufs=1, space="PSUM"))
        xs_b16 = [xpp.tile([P, L], BF16, name=f"xs_b16{d}")
                  for d in range(2)]
        for d in range(2):
            act.copy(xs_b16[d][:], xs_act[d][:])
        for half in range(2):
            o = half * H
            dbc_sb = xpp.tile([RK + 2 * N, H], BF16, name="dbc_sb", bufs=2)
            for j in range(2):
                ps = xpps.tile([RK + 2 * N, LB], F32, name="dbcps")
                for d in range(2):
                    pe.matmul(out=ps[:],
                              lhsT=wxp_sb[:, d * 64:(d + 1) * 64],
                              rhs=xs_b16[d][:, o + j * LB:o + (j + 1) * LB],
                              start=(d == 0), stop=(d == 1))
                act.copy(dbc_sb[:, j * LB:(j + 1) * LB], ps[:])
            sync.dma_start(out=dbc_in_h[half][:], in_=dbc_sb[:])
            gps.collective_compute("AllReduce", OP.add,
                                   replica_groups=GRP_B,
                                   ins=[dbc_in_h[half].opt()],
                                   outs=[dbc_out_h[half].opt()])
            dtr = xpp.tile([RK, H], BF16, name="dtr", bufs=2)
            sync.dma_start(out=dtr[:], in_=dbc_out_h[half][0:RK, :])
            for d in range(2):
                for j in range(2):
                    ps = dtps.tile([P, LB], F32, name="dtps")
                    pe.matmul(out=ps[:],
                              lhsT=wdt_sb[:, d * P:(d + 1) * P],
                              rhs=dtr[:, j * LB:(j + 1) * LB],
                              start=True, stop=True)
                    # softplus(x) = ln(1 + exp(x)) (no softplus ACT table)
                    et = xpp.tile([P, LB], F32, name="et", bufs=2)
                    act.activation(out=et[:], in_=ps[:], func=AF.Exp,
                                   bias=dtb_sb[:, d:d + 1])
                    act.activation(
                        out=dtT[d][:, o + j * LB:o + (j + 1) * LB],
                        in_=et[:], func=AF.Ln, bias=1.0)
                vec.tensor_tensor(out=vT[d][:, o:o + H],
                                  in0=dtT[d][:, o:o + H],
                                  in1=xs_act[d][:, o:o + H], op=OP.mult)

        # =============================================================
        # Phase 7-9, pipelined over L-halves:
        #   per half: per (n, d): dA = exp(A[:,n]*dt); dBx = v*Bm_n;
        #   h = scan(dA, dBx) [state handoff between halves];
        #   prod = h*Cm_n; PE identity-matmul accumulates sum_n in PSUM.
        #   Then gate + out_proj + ReduceScatter + residual for the half,
        #   overlapping the other half\'s scan on the vector engine.
        # =============================================================
        with tc.tile_pool(name="bmb", bufs=3) as bmbp, \
             tc.tile_pool(name="cmb", bufs=3) as cmbp, \
             tc.tile_pool(name="sca", bufs=2) as scap, \
             tc.tile_pool(name="scb", bufs=3) as scbp, \
             tc.tile_pool(name="sch", bufs=2) as schp, \
             tc.tile_pool(name="ygpool", bufs=1) as ygp, \
             tc.tile_pool(name="fin", bufs=1) as finp, \
             tc.tile_pool(name="ypsum", bufs=1, space="PSUM") as ypsp, \
             tc.tile_pool(name="atpsum", bufs=2, space="PSUM") as atps:
            hfin = stile([P, 2 * N], F32, "hfin")
            yg = [ygp.tile([P, L], BF16, name=f"yg{d}") for d in range(2)]
            att_sb = finp.tile([CT, L], BF16, name="att_sb")
            out_sb = finp.tile([CT, L], F32, name="out_sb")
            for half in range(2):
                o = half * H
                yps = [[ypsp.tile([P, LB], F32, name=f"yps{d}_{j}",
                                  tag=f"yps{d}_{j}") for j in range(2)]
                       for d in range(2)]
                for n in range(N):
                    bc = bmbp.tile([P, 2 * H], BF16, name="bc")
                    brow = cmbp.tile([1, 2 * H], BF16, name="brow", bufs=2)
                    sync.dma_start(out=brow[:, 0:H],
                                   in_=dbc_out_h[half][RK + n:RK + n + 1, :])
                    sync.dma_start(
                        out=brow[:, H:2 * H],
                        in_=dbc_out_h[half][RK + N + n:RK + N + n + 1, :])
                    gps.partition_broadcast(bc[:], brow[:])
                    bmb = bc[:, 0:H]
                    cmb = bc[:, H:2 * H]
                    for d in range(2):
                        idx = n * 2 + d
                        da = scap.tile([P, H], BF16, name="da")
                        dbx = scbp.tile([P, H], BF16, name="dbx")
                        hs = schp.tile([P, H], BF16, name="hs")
                        act.activation(
                            out=da[:], in_=dtT[d][:, o:o + H], func=AF.Exp,
                            scale=acol_sb[:, d * N + n:d * N + n + 1])
                        vec.tensor_tensor(out=dbx[:], in0=vT[d][:, o:o + H],
                                          in1=bmb, op=OP.mult)
                        vec.tensor_tensor_scan(
                            out=hs[:], data0=da[:], data1=dbx[:],
                            initial=(0.0 if half == 0
                                     else hfin[:, idx:idx + 1]),
                            op0=OP.mult, op1=OP.add)
                        if half == 0:
                            act.copy(hfin[:, idx:idx + 1], hs[:, H - 1:H])
                        vec.tensor_tensor(out=dbx[:], in0=hs[:], in1=cmb,
                                          op=OP.mult)
                        for j in range(2):
                            pe.matmul(out=yps[d][j][:], lhsT=idt_sb[:],
                                      rhs=dbx[:, j * LB:(j + 1) * LB],
                                      start=(n == 0), stop=(n == N - 1),
                                      skip_group_check=True)

                # ---- evacuate + gate + out_proj + RS for this half ----
                for d in range(2):
                    for j in range(2):
                        glb = half * 2 + j
                        act.copy(yacc[d][:, glb * LB:(glb + 1) * LB],
                                 yps[d][j][:])
                    vec.scalar_tensor_tensor(
                        out=yacc[d][:, o:o + H], in0=xs_act[d][:, o:o + H],
                        scalar=dskip_sb[:, d:d + 1], in1=yacc[d][:, o:o + H],
                        op0=OP.mult, op1=OP.add)
                    vec.tensor_tensor(out=yg[d][:, o:o + H],
                                      in0=yacc[d][:, o:o + H],
                                      in1=zsil[d][:, o:o + H], op=OP.mult)
                att_h = att_in0 if half == 0 else att_in1
                for j in range(2):
                    glb = half * 2 + j
                    for ct in range(4):
                        ps = atps.tile([P, LB], F32, name="atps")
                        for d in range(2):
                            pe.matmul(
                                out=ps[:],
                                lhsT=wout_sb[:, (d * 4 + ct) * P:(d * 4 + ct + 1) * P],
                                rhs=yg[d][:, glb * LB:(glb + 1) * LB],
                                start=(d == 0), stop=(d == 1))
                        st = ygp.tile([P, LB], BF16, name="atstage", bufs=4)
                        act.copy(st[:], ps[:])
                        sync.dma_start(
                            out=att_h[ct * P:(ct + 1) * P,
                                      j * LB:(j + 1) * LB],
                            in_=st[:])
                ao = att_out0 if half == 0 else att_out1
                gps.collective_compute("ReduceScatter", OP.add,
                                       replica_groups=GRP_B,
                                       ins=[(att_in0 if half == 0
                                             else att_in1).opt()],
                                       outs=[ao.opt()])
                hs_ = slice(o, o + H)
                sync.dma_start(out=att_sb[:, hs_], in_=ao[:])
                vec.tensor_tensor(out=out_sb[:, hs_], in0=att_sb[:, hs_],
                                  in1=ff_out[:, hs_], op=OP.add)
                vec.tensor_tensor(out=out_sb[:, hs_], in0=out_sb[:, hs_],
                                  in1=xct_sb[:, hs_], op=OP.add)
                sync.dma_start(out=out.ap()[:, hs_], in_=out_sb[:, hs_])

        xpp_ctx.close()
    _pers_ctx.close()


_NC_CACHE = None
LAST_EXEC_NS = None


def _get_nc():
    global _NC_CACHE
    if _NC_CACHE is None:
        _NC_CACHE = build_graph()
    return _NC_CACHE


def make_in_maps(inputs):
    f32 = lambda a: np.ascontiguousarray(np.asarray(a), dtype=np.float32)
    bf16 = lambda a: np.ascontiguousarray(
        np.asarray(a, dtype=np.float32).astype(ml_dtypes.bfloat16))
    x = f32(inputs["x"])
    ff_w = f32(inputs["ff_w"])
    ff_b = f32(inputs["ff_b"])
    g = f32(inputs["bn_gamma"])
    bt = f32(inputs["bn_beta"])
    w_in = f32(inputs["w_in"])
    conv_w = f32(inputs["conv_w"])
    conv_b = f32(inputs["conv_b"])
    w_xproj = f32(inputs["w_xproj"])
    w_dt = f32(inputs["w_dt"])
    dt_bias = f32(inputs["dt_bias"])
    A = -np.exp(f32(inputs["A_log"]))
    D_skip = f32(inputs["D_skip"])
    w_out = f32(inputs["w_out"])
    ffw_t = np.transpose(ff_w, (2, 1, 0))  # [K, C, co]

    in_maps = []
    for i in range(8):
        b, q = i // 4, i % 4
        dsl = slice(q * S, (q + 1) * S)
        csl = slice(q * CT, (q + 1) * CT)
        in_maps.append({
            "xb": bf16(x[b]),
            "xct": f32(x[b, csl]),
            "ffw": bf16(ffw_t[:, :, csl]),
            "ffb": f32(ff_b[csl].reshape(CT, 1)),
            "gamma": f32(g[csl].reshape(CT, 1)),
            "beta": f32(bt[csl].reshape(CT, 1)),
            "winx": bf16(w_in[:, :DI][:, dsl]),
            "winz": bf16(w_in[:, DI:][:, dsl]),
            "convd": bf16(np.stack([
                np.stack([np.diag(conv_w[dsl][dd * P:(dd + 1) * P, k])
                          for dd in range(2)]).reshape(S, P)
                for k in range(DC)])),
            "convb": f32(conv_b[dsl].reshape(S, 1)),
            "wxp": bf16(w_xproj[dsl]),
            "wdt": bf16(w_dt[:, dsl]),
            "dtb": f32(dt_bias[dsl].reshape(S, 1)),
            "acol": f32(A[dsl]),
            "dskip": f32(D_skip[dsl].reshape(S, 1)),
            "wout": bf16(w_out[dsl]),
            "ident": np.eye(P, dtype=np.float32).astype(ml_dtypes.bfloat16),
        })
    return in_maps


def _install_ntff_hook():
    """The agent image's antenv lacks axon_hooks; recreate it so
    run_bass_kernel_spmd(trace=True) can NTFF-profile via the axon .so."""
    import types
    if "antenv.axon_hooks" in sys.modules:
        return
    try:
        from trn_agent_boot.trn_boot import _ntff_profile_via_ctypes
        hook = _ntff_profile_via_ctypes("/opt/axon/libaxon_pjrt.so")
    except Exception:
        hook = None
    mod = types.ModuleType("antenv.axon_hooks")
    mod.get_axon_ntff_profile_hook = lambda: hook
    mod.set_axon_ntff_profile_hook = lambda h: None
    sys.modules["antenv.axon_hooks"] = mod


def kernel(**inputs):
    global LAST_EXEC_NS
    nc = _get_nc()
    in_maps = make_in_maps(inputs)
    trace = os.environ.get("KERNEL_TRACE", "0") == "1"
    if trace:
        _install_ntff_hook()
    try:
        res = run_bass_kernel_spmd(nc, in_maps, core_ids=list(range(8)),
                                   trace=trace)
    except Exception:
        if not trace:
            raise
        res = run_bass_kernel_spmd(nc, in_maps, core_ids=list(range(8)),
                                   trace=False)
    LAST_EXEC_NS = res.exec_time_ns
    out = np.empty((B, C, L), dtype=np.float32)
    for i in range(8):
        b, q = i // 4, i % 4
        out[b, q * CT:(q + 1) * CT] = res.results[i]["out"]
    return out


# revision 35
# speedup vs baseline: 1.1892x; 1.0376x over previous
"""Trainium2 8-core kernel for the ConvFF + BatchNorm + Mamba block.

Sharding (8 NeuronCores): core i -> b = i//4 (batch), q = i%4.
  - Front (ff conv + BN): computes output-channel tile q (128 of 512
    channels) for batch b. BN stats all-reduced across the b-pair.
  - normed all-gathered within each b-group of 4 cores.
  - Mamba: d_inner slice q (256 of 1024 channels) for batch b; the
    x_proj partial is all-reduced within the b-group; the selective
    scan runs fully local via the DVE tensor_tensor_scan instruction
    (h_t = dA_t * h_{t-1} + dBx_t along the free/time axis).
  - out_proj partials reduce-scattered within the b-group back to
    channel tile q; each core emits its [128, 2048] output shard.

Everything channel-major [channel, time] on-chip; no transposes.
"""

import os
import sys
import numpy as np

for _p in ("/opt/trn_rl_repo", "/root/.axon_site/_ro/trn_rl_repo"):
    if os.path.isdir(_p) and _p not in sys.path:
        sys.path.append(_p)

import ml_dtypes  # noqa: E402

from concourse import bass, bacc, mybir, tile  # noqa: E402
from concourse.bass_utils import run_bass_kernel_spmd  # noqa: E402

F32 = mybir.dt.float32
BF16 = mybir.dt.bfloat16
AF = mybir.ActivationFunctionType
OP = mybir.AluOpType

B, L, C, DI, N, RK, KK, DC = 2, 2048, 512, 1024, 16, 32, 7, 4
S = DI // 4      # 256 d_inner channels per core
CT = C // 4      # 128 output channels per core
P = 128
LB = 512         # l-block (one PSUM bank of f32)
NLB = L // LB
EPS = 1e-5

BCAST_DMA = os.environ.get("BCAST_DMA", "0") == "1"

GRP_B = [[0, 1, 2, 3], [4, 5, 6, 7]]        # b-groups
GRP_PAIR = [[0, 4], [1, 5], [2, 6], [3, 7]]  # same-ctile pairs for BN stats


def build_graph():
    nc = bacc.Bacc("TRN2", target_bir_lowering=False, debug=False,
                   num_devices=8)

    # ---- kernel I/O --------------------------------------------------
    xb = nc.dram_tensor("xb", [C, L], BF16, kind="ExternalInput")
    xct = nc.dram_tensor("xct", [CT, L], F32, kind="ExternalInput")
    ffw = nc.dram_tensor("ffw", [KK, C, CT], BF16, kind="ExternalInput")
    ffb = nc.dram_tensor("ffb", [CT, 1], F32, kind="ExternalInput")
    gamma = nc.dram_tensor("gamma", [CT, 1], F32, kind="ExternalInput")
    beta = nc.dram_tensor("beta", [CT, 1], F32, kind="ExternalInput")
    winx = nc.dram_tensor("winx", [C, S], BF16, kind="ExternalInput")
    winz = nc.dram_tensor("winz", [C, S], BF16, kind="ExternalInput")
    convd = nc.dram_tensor("convd", [DC, S, P], BF16, kind="ExternalInput")
    convb = nc.dram_tensor("convb", [S, 1], F32, kind="ExternalInput")
    wxp = nc.dram_tensor("wxp", [S, RK + 2 * N], BF16, kind="ExternalInput")
    wdt = nc.dram_tensor("wdt", [RK, S], BF16, kind="ExternalInput")
    dtb = nc.dram_tensor("dtb", [S, 1], F32, kind="ExternalInput")
    acol = nc.dram_tensor("acol", [S, N], F32, kind="ExternalInput")
    dskip = nc.dram_tensor("dskip", [S, 1], F32, kind="ExternalInput")
    wout = nc.dram_tensor("wout", [S, C], BF16, kind="ExternalInput")
    ident = nc.dram_tensor("ident", [P, P], BF16, kind="ExternalInput")
    out = nc.dram_tensor("out", [CT, L], F32, kind="ExternalOutput")

    with tile.TileContext(nc) as tc:
        _emit(nc, tc, xb, xct, ffw, ffb, gamma, beta, winx, winz, convd,
              convb, wxp, wdt, dtb, acol, dskip, wout, ident, out)

    nc.compile()
    return nc


def _emit(nc, tc, xb, xct, ffw, ffb, gamma, beta, winx, winz, convd, convb,
          wxp, wdt, dtb, acol, dskip, wout, ident, out):
    sync = nc.sync
    vec = nc.vector
    act = nc.scalar
    pe = nc.tensor
    gps = nc.gpsimd

    import contextlib
    _pers_ctx = contextlib.ExitStack()
    _pers = _pers_ctx.enter_context(tc.tile_pool(name="pers", bufs=1))

    def stile(shape, dtype, name):
        return _pers.tile(shape, dtype, name=name, tag=name)

    # ---- DRAM bounce buffers for collectives -------------------------
    with tc.tile_pool(name="dram", bufs=1, space="DRAM") as dram:
        bn_in = dram.tile([CT, 2], F32, name="bn_in")
        bn_out = dram.tile([CT, 2], F32, name="bn_out")
        ng_in0 = dram.tile([CT, L // 2], BF16, name="ng_in0")
        ng_in1 = dram.tile([CT, L // 2], BF16, name="ng_in1")
        ng_out0 = dram.tile([C, L // 2], BF16, name="ng_out0")
        ng_out1 = dram.tile([C, L // 2], BF16, name="ng_out1")
        dbc_in_h = [dram.tile([RK + 2 * N, L // 2], BF16, name=f"dbc_in{h}")
                    for h in range(2)]
        dbc_out_h = [dram.tile([RK + 2 * N, L // 2], BF16,
                               name=f"dbc_out{h}") for h in range(2)]
        att_in0 = dram.tile([C, L // 2], BF16, name="att_in0")
        att_in1 = dram.tile([C, L // 2], BF16, name="att_in1")
        att_out0 = dram.tile([CT, L // 2], BF16, name="att_out0")
        att_out1 = dram.tile([CT, L // 2], BF16, name="att_out1")

        # ---- persistent SBUF tiles ----------------------------------
        ffb_sb = stile([CT, 1], F32, "ffb_sb")
        gamma_sb = stile([CT, 1], F32, "gamma_sb")
        beta_sb = stile([CT, 1], F32, "beta_sb")
        wxp_sb = stile([P, 2 * (RK + 2 * N)], BF16, "wxp_sb")
        wdt_sb = stile([RK, S], BF16, "wdt_sb")
        dtb_sb = stile([P, 2], F32, "dtb_sb")
        acol_sb = stile([P, 2 * N], F32, "acol_sb")
        dskip_sb = stile([P, 2], F32, "dskip_sb")
        convd_sb = stile([P, DC * 2 * P], BF16, "convd_sb")
        convb_sb = stile([P, 2], F32, "convb_sb")
        wout_sb = stile([P, 8 * P], BF16, "wout_sb")
        xct_sb = stile([CT, L], F32, "xct_sb")

        ff_out = stile([CT, L], F32, "ff_out")
        xs_act = [stile([P, L], F32, f"xs_act{d}") for d in range(2)]
        zsil = [stile([P, L], F32, f"zsil{d}") for d in range(2)]
        dtT = [stile([P, L], F32, f"dtT{d}") for d in range(2)]
        vT = [stile([P, L], BF16, f"vT{d}") for d in range(2)]
        idt_sb = stile([P, P], BF16, "idt_sb")
        yacc = [stile([P, L], F32, f"yacc{d}") for d in range(2)]

        sync.dma_start(out=ffb_sb[:], in_=ffb.ap()[:, :])

        # =============================================================
        # Phase 1: ff conv (Conv1d k=7 same-pad) + ReLU -> ff_out
        # =============================================================
        with tc.tile_pool(name="ffpool", bufs=1) as ffp, \
             tc.tile_pool(name="ffpsum", bufs=3, space="PSUM") as ffps:
            ffw_sb = ffp.tile([P, KK * 4 * P], BF16, name="ffw_sb")
            for ci in range(4):
                eng = (act, gps, sync, act)[ci]
                eng.dma_start(
                    out=ffw_sb[:, ci * KK * P:(ci + 1) * KK * P]
                    .rearrange("p (k m) -> p k m", k=KK),
                    in_=ffw.ap()[:, ci * P:(ci + 1) * P, :]
                    .rearrange("k p m -> p k m"))
            x_sb = []
            for ci in range(4):
                t = ffp.tile([P, L + 6], BF16, name=f"x_sb{ci}")
                gps.memset(t[:, 0:3], 0.0)
                gps.memset(t[:, L + 3:L + 6], 0.0)
                eng = (sync, act, gps, sync)[ci]
                eng.dma_start(out=t[:, 3:3 + L // 2],
                              in_=xb.ap()[ci * P:(ci + 1) * P, 0:L // 2])
                eng.dma_start(out=t[:, 3 + L // 2:3 + L],
                              in_=xb.ap()[ci * P:(ci + 1) * P, L // 2:L])
                x_sb.append(t)

            for lb in range(NLB):
                ps = ffps.tile([P, LB], F32, name="ffps")
                nmm = KK * 4
                j = 0
                for k in range(KK):
                    for ci in range(4):
                        jj = ci * KK + k
                        pe.matmul(
                            out=ps[:],
                            lhsT=ffw_sb[:, jj * P:(jj + 1) * P],
                            rhs=x_sb[ci][:, k + lb * LB:k + lb * LB + LB],
                            start=(j == 0), stop=(j == nmm - 1))
                        j += 1
                act.activation(out=ff_out[:, lb * LB:(lb + 1) * LB], in_=ps[:],
                               func=AF.Relu, bias=ffb_sb[:, 0:1])

        sync.dma_start(out=gamma_sb[:], in_=gamma.ap()[:, :])
        sync.dma_start(out=beta_sb[:], in_=beta.ap()[:, :])
        sync.dma_start(out=wdt_sb[:], in_=wdt.ap()[:, :])
        sync.dma_start(out=xct_sb[:], in_=xct.ap()[:, :])
        sync.dma_start(out=idt_sb[:], in_=ident.ap()[:, :])
        for d in range(2):
            rs = slice(d * P, (d + 1) * P)
            sync.dma_start(out=wxp_sb[:, d * 64:(d + 1) * 64],
                           in_=wxp.ap()[rs, :])
            sync.dma_start(out=dtb_sb[:, d:d + 1], in_=dtb.ap()[rs, :])
            sync.dma_start(out=acol_sb[:, d * N:(d + 1) * N],
                           in_=acol.ap()[rs, :])
            sync.dma_start(out=dskip_sb[:, d:d + 1], in_=dskip.ap()[rs, :])
            sync.dma_start(out=convb_sb[:, d:d + 1], in_=convb.ap()[rs, :])
            sync.dma_start(out=wout_sb[:, d * 4 * P:(d + 1) * 4 * P],
                           in_=wout.ap()[rs, :])
            sync.dma_start(
                out=convd_sb[:, d * DC * P:(d + 1) * DC * P]
                .rearrange("p (k m) -> p k m", k=DC),
                in_=convd.ap()[:, d * P:(d + 1) * P, :]
                .rearrange("k p m -> p k m"))

        # =============================================================
        # Phase 2: BN stats + pairwise AllReduce + normalize (-> bf16)
        # =============================================================
        with tc.tile_pool(name="bnpool", bufs=1) as bnp:
            stat = bnp.tile([CT, 2], F32, name="stat")
            stat2 = bnp.tile([CT, 2], F32, name="stat2")
            sq = bnp.tile([CT, L], BF16, name="sq")
            vec.tensor_reduce(out=stat[:, 0:1], in_=ff_out[:],
                              axis=mybir.AxisListType.X, op=OP.add)
            act.activation(out=sq[:], in_=ff_out[:], func=AF.Square,
                           accum_out=stat[:, 1:2])
            sync.dma_start(out=bn_in[:], in_=stat[:])
            gps.collective_compute("AllReduce", OP.add,
                                   replica_groups=GRP_PAIR,
                                   ins=[bn_in.opt()], outs=[bn_out.opt()])
            sync.dma_start(out=stat2[:], in_=bn_out[:])

            mu = bnp.tile([CT, 1], F32, name="mu")
            ex2 = bnp.tile([CT, 1], F32, name="ex2")
            msq = bnp.tile([CT, 1], F32, name="msq")
            var = bnp.tile([CT, 1], F32, name="var")
            std = bnp.tile([CT, 1], F32, name="std")
            rstd = bnp.tile([CT, 1], F32, name="rstd")
            bscale = bnp.tile([CT, 1], F32, name="bscale")
            tmp1 = bnp.tile([CT, 1], F32, name="tmp1")
            bshift = bnp.tile([CT, 1], F32, name="bshift")
            act.mul(mu[:], stat2[:, 0:1], 1.0 / (B * L))
            act.mul(ex2[:], stat2[:, 1:2], 1.0 / (B * L))
            act.square(msq[:], mu[:])
            vec.tensor_tensor(out=var[:], in0=ex2[:], in1=msq[:],
                              op=OP.subtract)
            vec.tensor_scalar_add(out=var[:], in0=var[:], scalar1=EPS)
            act.activation(out=std[:], in_=var[:], func=AF.Sqrt)
            vec.reciprocal(rstd[:], std[:])
            vec.tensor_tensor(out=bscale[:], in0=rstd[:], in1=gamma_sb[:],
                              op=OP.mult)
            vec.tensor_tensor(out=tmp1[:], in0=mu[:], in1=bscale[:],
                              op=OP.mult)
            vec.tensor_tensor(out=bshift[:], in0=beta_sb[:], in1=tmp1[:],
                              op=OP.subtract)

            nrm_t = bnp.tile([CT, L], F32, name="nrm_t")
            nrm_l = bnp.tile([CT, L], BF16, name="nrm_l")
            act.activation(out=nrm_t[:], in_=ff_out[:], func=AF.Copy,
                           scale=bscale[:, 0:1])
            vec.tensor_scalar_add(out=nrm_l[:], in0=nrm_t[:],
                                  scalar1=bshift[:, 0:1])
            sync.dma_start(out=ng_in0[:], in_=nrm_l[:, 0:L // 2])
            sync.dma_start(out=ng_in1[:], in_=nrm_l[:, L // 2:L])

        # =============================================================
        # Phase 3: AllGather normed within b-group
        # =============================================================
        gps.collective_compute("AllGather", OP.bypass,
                               replica_groups=GRP_B,
                               ins=[ng_in0.opt()], outs=[ng_out0.opt()])
        gps.collective_compute("AllGather", OP.bypass,
                               replica_groups=GRP_B,
                               ins=[ng_in1.opt()], outs=[ng_out1.opt()])
        H = L // 2

        # =============================================================
        # Phase 4: in_proj (xs & z), depthwise conv, silu
        # =============================================================
        with tc.tile_pool(name="ippool", bufs=1) as ipp, \
             tc.tile_pool(name="ippsum", bufs=2, space="PSUM") as ipps, \
             tc.tile_pool(name="xsppool", bufs=1) as xspp:
            nrm_h = [[], []]
            for h, ngo in ((0, ng_out0), (1, ng_out1)):
                for ci in range(4):
                    t = ipp.tile([P, H], BF16, name=f"nrm{h}_{ci}")
                    sync.dma_start(out=t[:], in_=ngo[ci * P:(ci + 1) * P, :])
                    nrm_h[h].append(t)
            winx_sb = ipp.tile([P, 8 * P], BF16, name="winx_sb")
            winz_sb = ipp.tile([P, 8 * P], BF16, name="winz_sb")
            for ci in range(4):
                sync.dma_start(out=winx_sb[:, ci * 2 * P:(ci + 1) * 2 * P],
                               in_=winx.ap()[ci * P:(ci + 1) * P, :])
                sync.dma_start(out=winz_sb[:, ci * 2 * P:(ci + 1) * 2 * P],
                               in_=winz.ap()[ci * P:(ci + 1) * P, :])

            xsp = [xspp.tile([P, L + 3], BF16, name=f"xsp{d}")
                   for d in range(2)]
            for d in range(2):
                gps.memset(xsp[d][:, 0:3], 0.0)

            for lb in range(NLB):
                hh, loc = lb // 2, (lb % 2) * LB
                for d in range(2):
                    ps = ipps.tile([P, LB], F32, name="xzps")
                    for ci in range(4):
                        pe.matmul(out=ps[:],
                                  lhsT=winx_sb[:, (ci * 2 + d) * P:(ci * 2 + d + 1) * P],
                                  rhs=nrm_h[hh][ci][:, loc:loc + LB],
                                  start=(ci == 0), stop=(ci == 3))
                    act.copy(xsp[d][:, 3 + lb * LB:3 + (lb + 1) * LB], ps[:])
                    ps2 = ipps.tile([P, LB], F32, name="zps")
                    for ci in range(4):
                        pe.matmul(out=ps2[:],
                                  lhsT=winz_sb[:, (ci * 2 + d) * P:(ci * 2 + d + 1) * P],
                                  rhs=nrm_h[hh][ci][:, loc:loc + LB],
                                  start=(ci == 0), stop=(ci == 3))
                    act.activation(out=zsil[d][:, lb * LB:(lb + 1) * LB],
                                   in_=ps2[:], func=AF.Silu)

            # depthwise causal conv: 4 diagonal matmuls per (d, lb)
            with tc.tile_pool(name="cvpsum", bufs=3, space="PSUM") as cvps:
                for d in range(2):
                    for lb in range(NLB):
                        ps3 = cvps.tile([P, LB], F32, name="cvps")
                        for k in range(DC):
                            jj = d * DC + k
                            pe.matmul(
                                out=ps3[:],
                                lhsT=convd_sb[:, jj * P:(jj + 1) * P],
                                rhs=xsp[d][:, k + lb * LB:k + lb * LB + LB],
                                start=(k == 0), stop=(k == DC - 1))
                        act.activation(out=xs_act[d][:, lb * LB:(lb + 1) * LB],
                                       in_=ps3[:], func=AF.Silu,
                                       bias=convb_sb[:, d:d + 1])

        # =============================================================
        # Phase 5+6 (per L-half, overlapping the other half\'s scan):
        # x_proj partial + AllReduce -> dt_raw/Bm/Cm; dt = softplus; v
        # =============================================================
        xpp_ctx = contextlib.ExitStack()
        xpp = xpp_ctx.enter_context(tc.tile_pool(name="xppool", bufs=1))
        xpps = xpp_ctx.enter_context(
            tc.tile_pool(name="xppsum", bufs=1, space="PSUM"))
        dtps = xpp_ctx.enter_context(
            tc.tile_pool(name="dtpsum", bufs=1, space="PSUM"))
        xs_b16 = [xpp.tile([P, L], BF16, name=f"xs_b16{d}")
                  for d in range(2)]
        for d in range(2):
            act.copy(xs_b16[d][:], xs_act[d][:])
        for half in range(2):
            o = half * H
            dbc_sb = xpp.tile([RK + 2 * N, H], BF16, name="dbc_sb", bufs=2)
            for j in range(2):
                ps = xpps.tile([RK + 2 * N, LB], F32, name="dbcps")
                for d in range(2):
                    pe.matmul(out=ps[:],
                              lhsT=wxp_sb[:, d * 64:(d + 1) * 64],
                              rhs=xs_b16[d][:, o + j * LB:o + (j + 1) * LB],
                              start=(d == 0), stop=(d == 1))
                act.copy(dbc_sb[:, j * LB:(j + 1) * LB], ps[:])
            sync.dma_start(out=dbc_in_h[half][:], in_=dbc_sb[:])
            gps.collective_compute("AllReduce", OP.add,
                                   replica_groups=GRP_B,
                                   ins=[dbc_in_h[half].opt()],
                                   outs=[dbc_out_h[half].opt()])
            dtr = xpp.tile([RK, H], BF16, name="dtr", bufs=2)
            sync.dma_start(out=dtr[:], in_=dbc_out_h[half][0:RK, :])
            for d in range(2):
                for j in range(2):
                    ps = dtps.tile([P, LB], F32, name="dtps")
                    pe.matmul(out=ps[:],
                              lhsT=wdt_sb[:, d * P:(d + 1) * P],
                              rhs=dtr[:, j * LB:(j + 1) * LB],
                              start=True, stop=True)
                    # softplus(x) = ln(1 + exp(x)) (no softplus ACT table)
                    et = xpp.tile([P, LB], F32, name="et", bufs=2)
                    act.activation(out=et[:], in_=ps[:], func=AF.Exp,
                                   bias=dtb_sb[:, d:d + 1])
                    act.activation(
                        out=dtT[d][:, o + j * LB:o + (j + 1) * LB],
                        in_=et[:], func=AF.Ln, bias=1.0)
                vec.tensor_tensor(out=vT[d][:, o:o + H],
                                  in0=dtT[d][:, o:o + H],
                                  in1=xs_act[d][:, o:o + H], op=OP.mult)

        # =============================================================
        # Phase 7-9, pipelined over L-halves:
        #   per half: per (n, d): dA = exp(A[:,n]*dt); dBx = v*Bm_n;
        #   h = scan(dA, dBx) [state handoff between halves];
        #   prod = h*Cm_n; PE identity-matmul accumulates sum_n in PSUM.
        #   Then gate + out_proj + ReduceScatter + residual for the half,
        #   overlapping the other half\'s scan on the vector engine.
        # =============================================================
        with tc.tile_pool(name="bmb", bufs=3) as bmbp, \
             tc.tile_pool(name="cmb", bufs=3) as cmbp, \
             tc.tile_pool(name="sca", bufs=2) as scap, \
             tc.tile_pool(name="scb", bufs=3) as scbp, \
             tc.tile_pool(name="sch", bufs=2) as schp, \
             tc.tile_pool(name="ygpool", bufs=1) as ygp, \
             tc.tile_pool(name="fin", bufs=1) as finp, \
             tc.tile_pool(name="ypsum", bufs=1, space="PSUM") as ypsp, \
             tc.tile_pool(name="atpsum", bufs=2, space="PSUM") as atps:
            hfin = stile([P, 2 * N], F32, "hfin")
            yg = [ygp.tile([P, L], BF16, name=f"yg{d}") for d in range(2)]
            att_sb = finp.tile([CT, L], BF16, name="att_sb")
            out_sb = finp.tile([CT, L], F32, name="out_sb")
            for half in range(2):
                o = half * H
                yps = [[ypsp.tile([P, LB], F32, name=f"yps{d}_{j}",
                                  tag=f"yps{d}_{j}") for j in range(2)]
                       for d in range(2)]
                for n in range(N):
                    bc = bmbp.tile([P, 2 * H], BF16, name="bc")
                    brow = cmbp.tile([1, 2 * H], BF16, name="brow", bufs=2)
                    sync.dma_start(out=brow[:, 0:H],
                                   in_=dbc_out_h[half][RK + n:RK + n + 1, :])
                    sync.dma_start(
                        out=brow[:, H:2 * H],
                        in_=dbc_out_h[half][RK + N + n:RK + N + n + 1, :])
                    gps.partition_broadcast(bc[:], brow[:])
                    bmb = bc[:, 0:H]
                    cmb = bc[:, H:2 * H]
                    for d in range(2):
                        idx = n * 2 + d
                        da = scap.tile([P, H], BF16, name="da")
                        dbx = scbp.tile([P, H], BF16, name="dbx")
                        hs = schp.tile([P, H], BF16, name="hs")
                        act.activation(
                            out=da[:], in_=dtT[d][:, o:o + H], func=AF.Exp,
                            scale=acol_sb[:, d * N + n:d * N + n + 1])
                        vec.tensor_tensor(out=dbx[:], in0=vT[d][:, o:o + H],
                                          in1=bmb, op=OP.mult)
                        vec.tensor_tensor_scan(
                            out=hs[:], data0=da[:], data1=dbx[:],
                            initial=(0.0 if half == 0
                                     else hfin[:, idx:idx + 1]),
                            op0=OP.mult, op1=OP.add)
                        if half == 0:
                            act.copy(hfin[:, idx:idx + 1], hs[:, H - 1:H])
                        vec.tensor_tensor(out=dbx[:], in0=hs[:], in1=cmb,
                                          op=OP.mult)
                        for j in range(2):
                            pe.matmul(out=yps[d][j][:], lhsT=idt_sb[:],
                                      rhs=dbx[:, j * LB:(j + 1) * LB],
                                      start=(n == 0), stop=(n == N - 1),
                                      skip_group_check=True)

                # ---- evacuate + gate + out_proj + RS for this half ----
                for d in range(2):
                    for j in range(2):
                        glb = half * 2 + j
                        act.copy(yacc[d][:, glb * LB:(glb + 1) * LB],
                                 yps[d][j][:])
                    vec.scalar_tensor_tensor(
                        out=yacc[d][:, o:o + H], in0=xs_act[d][:, o:o + H],
                        scalar=dskip_sb[:, d:d + 1], in1=yacc[d][:, o:o + H],
                        op0=OP.mult, op1=OP.add)
                    vec.tensor_tensor(out=yg[d][:, o:o + H],
                                      in0=yacc[d][:, o:o + H],
                                      in1=zsil[d][:, o:o + H], op=OP.mult)
                att_h = att_in0 if half == 0 else att_in1
                for j in range(2):
                    glb = half * 2 + j
                    for ct in range(4):
                        ps = atps.tile([P, LB], F32, name="atps")
                        for d in range(2):
                            pe.matmul(
                                out=ps[:],
                                lhsT=wout_sb[:, (d * 4 + ct) * P:(d * 4 + ct + 1) * P],
                                rhs=yg[d][:, glb * LB:(glb + 1) * LB],
                                start=(d == 0), stop=(d == 1))
                        st = ygp.tile([P, LB], BF16, name="atstage", bufs=4)
                        act.copy(st[:], ps[:])
                        sync.dma_start(
                            out=att_h[ct * P:(ct + 1) * P,
                                      j * LB:(j + 1) * LB],
                            in_=st[:])
                ao = att_out0 if half == 0 else att_out1
                gps.collective_compute("ReduceScatter", OP.add,
                                       replica_groups=GRP_B,
                                       ins=[(att_in0 if half == 0
                                             else att_in1).opt()],
                                       outs=[ao.opt()])
                hs_ = slice(o, o + H)
                sync.dma_start(out=att_sb[:, hs_], in_=ao[:])
                vec.tensor_tensor(out=out_sb[:, hs_], in0=att_sb[:, hs_],
                                  in1=ff_out[:, hs_], op=OP.add)
                vec.tensor_tensor(out=out_sb[:, hs_], in0=out_sb[:, hs_],
                                  in1=xct_sb[:, hs_], op=OP.add)
                sync.dma_start(out=out.ap()[:, hs_], in_=out_sb[:, hs_])

        xpp_ctx.close()
    _pers_ctx.close()


_NC_CACHE = None
LAST_EXEC_NS = None


def _get_nc():
    global _NC_CACHE
    if _NC_CACHE is None:
        _NC_CACHE = build_graph()
    return _NC_CACHE


def make_in_maps(inputs):
    f32 = lambda a: np.ascontiguousarray(np.asarray(a), dtype=np.float32)
    bf16 = lambda a: np.ascontiguousarray(
        np.asarray(a, dtype=np.float32).astype(ml_dtypes.bfloat16))
    x = f32(inputs["x"])
    ff_w = f32(inputs["ff_w"])
    ff_b = f32(inputs["ff_b"])
    g = f32(inputs["bn_gamma"])
    bt = f32(inputs["bn_beta"])
    w_in = f32(inputs["w_in"])
    conv_w = f32(inputs["conv_w"])
    conv_b = f32(inputs["conv_b"])
    w_xproj = f32(inputs["w_xproj"])
    w_dt = f32(inputs["w_dt"])
    dt_bias = f32(inputs["dt_bias"])
    A = -np.exp(f32(inputs["A_log"]))
    D_skip = f32(inputs["D_skip"])
    w_out = f32(inputs["w_out"])
    ffw_t = np.transpose(ff_w, (2, 1, 0))  # [K, C, co]

    in_maps = []
    for i in range(8):
        b, q = i // 4, i % 4
        dsl = slice(q * S, (q + 1) * S)
        csl = slice(q * CT, (q + 1) * CT)
        in_maps.append({
            "xb": bf16(x[b]),
            "xct": f32(x[b, csl]),
            "ffw": bf16(ffw_t[:, :, csl]),
            "ffb": f32(ff_b[csl].reshape(CT, 1)),
            "gamma": f32(g[csl].reshape(CT, 1)),
            "beta": f32(bt[csl].reshape(CT, 1)),
            "winx": bf16(w_in[:, :DI][:, dsl]),
            "winz": bf16(w_in[:, DI:][:, dsl]),
            "convd": bf16(np.stack([
                np.stack([np.diag(conv_w[dsl][dd * P:(dd + 1) * P, k])
                          for dd in range(2)]).reshape(S, P)
                for k in range(DC)])),
            "convb": f32(conv_b[dsl].reshape(S, 1)),
            "wxp": bf16(w_xproj[dsl]),
            "wdt": bf16(w_dt[:, dsl]),
            "dtb": f32(dt_bias[dsl].reshape(S, 1)),
            "acol": f32(A[dsl]),
            "dskip": f32(D_skip[dsl].reshape(S, 1)),
            "wout": bf16(w_out[dsl]),
            "ident": np.eye(P, dtype=np.float32).astype(ml_dtypes.bfloat16),
        })
    return in_maps


def _install_ntff_hook():
    """The agent image's antenv lacks axon_hooks; recreate it so
    run_bass_kernel_spmd(trace=True) can NTFF-profile via the axon .so."""
    import types
    if "antenv.axon_hooks" in sys.modules:
        return
    try:
        from trn_agent_boot.trn_boot import _ntff_profile_via_ctypes
        hook = _ntff_profile_via_ctypes("/opt/axon/libaxon_pjrt.so")
    except Exception:
        hook = None
    mod = types.ModuleType("antenv.axon_hooks")
    mod.get_axon_ntff_profile_hook = lambda: hook
    mod.set_axon_ntff_profile_hook = lambda h: None
    sys.modules["antenv.axon_hooks"] = mod


def kernel(**inputs):
    global LAST_EXEC_NS
    nc = _get_nc()
    in_maps = make_in_maps(inputs)
    trace = os.environ.get("KERNEL_TRACE", "0") == "1"
    if trace:
        _install_ntff_hook()
    try:
        res = run_bass_kernel_spmd(nc, in_maps, core_ids=list(range(8)),
                                   trace=trace)
    except Exception:
        if not trace:
            raise
        res = run_bass_kernel_spmd(nc, in_maps, core_ids=list(range(8)),
                                   trace=False)
    LAST_EXEC_NS = res.exec_time_ns
    out = np.empty((B, C, L), dtype=np.float32)
    for i in range(8):
        b, q = i // 4, i % 4
        out[b, q * CT:(q + 1) * CT] = res.results[i]["out"]
    return out
